# revision 18
# baseline (speedup 1.0000x reference)
"""Trainium2 Bass kernel for nn_Blur (upfirdn2d 4x4 blur, pad=(2,1)).

Formulation: out[i,j] = sum_{p,q} Kf[p,q] * x[i+p-2, j+q-2]   (Kf = flip(kernel2d))

For each W-tap q (4 taps), the H-convolution is a banded 64x64 matrix
Aq[i,h] = Kf[h-i+2, q].  Tolerance is 2e-2, so x streams as a single bf16
(the {1,3,9}/64 blur weights have <=4 mantissa bits: every bf16 product is
exact in fp32; end-to-end error ~5e-3) -- half the HBM traffic of an
fp32-faithful hi/lo split.

W-taps are fused in PAIRS into the K=128 contraction: x rows live in
partitions 0-63 (16 images x 64 cols, tight 128B stride -- the PE rhs
fetcher needs the power-of-two group stride to stream at 1 col/cycle);
one SBUF->SBUF queue DMA writes the same rows shifted left one column
into partitions 64-127.  Then
  pair(2,3): lhsT=[A2^T;A3^T], rhs c=0..62, out j=c   (tap3 reads dup)
  pair(0,1): lhsT=[A0^T;A1^T], rhs c=0..61, out j=c+2 (tap1 reads dup)
plus two N=1/image boundary matmuls (tap2@j=63, tap1@j=1, K=64 on the x
rows).  PSUM per-element has_written semantics: the FIRST matmul per
column group carries the only start=True (clearing the has-written state
across its partition range); every later matmul uses start=False, which
stores where clear and accumulates where set.  Two column groups run
concurrently on disjoint PE columns (tile_position (0,0)/(0,64)).
Tensor cost: ~1020 streamed cols/batch vs 2016 for a 4-tap scheme.

Input DMAs are issued 3 batches ahead on the sync queue so out-DMA
copy-waits never starve input issue (a stall >1.5us demotes the PE
clock-boost (HAM) to 1.2 GHz, which never recovers at <3.4us of
sustained activity).  The fp32 PSUM result is copied to SBUF as bf16
(alternating vector/scalar), DMA'd back, and cast to f32 on the host.
HBM per core: 8.4 MB in + 8.4 MB out = the ~47us roofline.

Sharding: the 16*512 = 8192 independent (n,c) images are split into 8
contiguous slabs of 1024 images, one per NeuronCore (data-parallel).
"""

import ml_dtypes
import numpy as np

import concourse.bacc as bacc
import concourse.bass as bass
import concourse.mybir as mybir
import concourse.tile as tile
from concourse.bass_utils import run_bass_kernel_spmd

N_CORES = 8
IMG = 64                      # H = W
N_IMAGES = 16 * 512           # 8192
PER_CORE = N_IMAGES // N_CORES  # 1024
GROUP = 16                    # images per batch
N_BATCH = PER_CORE // GROUP   # 64
TILE_W = GROUP * IMG          # 1024 free cols, 128B image stride
DT = mybir.dt.float32
IN_DT = mybir.dt.bfloat16
NP_IN = ml_dtypes.bfloat16

LAST_RESULTS = None  # BassKernelResults of the most recent run (for test.py)


def _build_weights(kernel2d: np.ndarray) -> np.ndarray:
    """[128, 256] bf16: cols 0:64=[A2^T;A3^T], 64:128=[A0^T;A1^T],
    128:192=[A2^T;0] (tap2 boundary), 192:256=[A1^T;0] (tap1 boundary)."""
    kf = np.flip(np.asarray(kernel2d, dtype=np.float64), (0, 1))
    a = np.zeros((4, IMG, IMG), dtype=np.float64)
    for q in range(4):
        for i in range(IMG):
            for p in range(4):
                h = i + p - 2
                if 0 <= h < IMG:
                    a[q, i, h] = kf[p, q]
    wts = np.zeros((128, 256), dtype=NP_IN)
    wts[:IMG, 0:IMG] = a[2].T.astype(NP_IN)
    wts[IMG:, 0:IMG] = a[3].T.astype(NP_IN)
    wts[:IMG, IMG:128] = a[0].T.astype(NP_IN)
    wts[IMG:, IMG:128] = a[1].T.astype(NP_IN)
    wts[:IMG, 128:192] = a[2].T.astype(NP_IN)
    wts[:IMG, 192:256] = a[1].T.astype(NP_IN)
    return wts


def _bass_module() -> bass.Bass:
    nc = bacc.Bacc(
        "TRN2",
        target_bir_lowering=False,
        debug=False,
        num_devices=N_CORES,
    )
    x_d = nc.dram_tensor("x", [N_BATCH, IMG, TILE_W], IN_DT, kind="ExternalInput")
    w_d = nc.dram_tensor("wts", [128, 256], IN_DT, kind="ExternalInput")
    o_d = nc.dram_tensor("out", [N_BATCH, 128, 512], IN_DT, kind="ExternalOutput")

    with tile.TileContext(nc) as tc:
        with (
            tc.tile_pool(name="const", bufs=1) as cpool,
            tc.tile_pool(name="inp", bufs=8) as ipool,
            tc.tile_pool(name="outp", bufs=8) as opool,
            tc.tile_pool(name="psum", bufs=8, space="PSUM") as ppool,
        ):
            w_tile = cpool.tile([128, 256], IN_DT)
            nc.sync.dma_start(w_tile[:], w_d[:])

            # HAM warmup: the PE clock-gate holds 1.2 GHz until ~3.4us of
            # sustained matmul activity.  Burn that window on dummy matmuls
            # (zeroed operands, result never read) that overlap the first
            # input DMA, so the real matmuls start at full clock.
            dummy = cpool.tile([128, 512], IN_DT, tag="warm_sbuf")
            nc.gpsimd.memset(dummy[:], 0.0)
            warm_ps = ppool.tile([128, 512], DT, tag="ps")
            for _ in range(12):
                nc.tensor.matmul(
                    warm_ps[:], dummy[:, 0:128], dummy[:], start=True, stop=True
                )

            def issue_in(b):
                t = ipool.tile([128, TILE_W], IN_DT)
                nc.sync.dma_start(t[0:IMG, :], x_d[b])
                in_tiles[b] = t

            def issue_dup(b):
                # shifted dup: partitions 64-127 = x rows one col left.
                # Per-image col 63 holds next-image garbage; no matmul
                # window ever reads it.  Issued IN_AHEAD batches after the
                # input DMA so its wait on in_b is already satisfied and
                # never blocks the input stream on the sync queue.
                t = in_tiles[b]
                nc.sync.dma_start(
                    t[IMG:128, 0 : TILE_W - 1], t[0:IMG, 1:TILE_W]
                )

            IN_AHEAD = 6
            in_tiles = {}
            for b in range(IN_AHEAD):
                issue_in(b)
            issue_dup(0)
            issue_dup(1)

            for b in range(N_BATCH):
                if b + IN_AHEAD < N_BATCH:
                    issue_in(b + IN_AHEAD)
                if b + 2 < N_BATCH:
                    issue_dup(b + 2)
                in_tile = in_tiles.pop(b)
                rhs3 = in_tile[:].rearrange("p (g w) -> p g w", w=IMG)

                ps = ppool.tile([128, 512], DT)
                out3s = [
                    ps[cg * IMG : (cg + 1) * IMG, :].rearrange(
                        "p (g w) -> p g w", w=IMG
                    )
                    for cg in range(2)
                ]
                gss = [slice(cg * 8, (cg + 1) * 8) for cg in range(2)]
                # tap2 at j=63 goes FIRST: its start=True initializes the
                # per-element has_written state for the whole column group.
                for cg in range(2):
                    nc.tensor.matmul(
                        out3s[cg][:, :, IMG - 1 : IMG],
                        w_tile[0:IMG, 128:192],
                        rhs3[0:IMG, gss[cg], IMG - 1 : IMG],
                        start=True,
                        stop=False,
                        tile_position=(0, cg * IMG),
                        skip_group_check=True,
                    )
                # pair(2,3): stores into cleared elements j=0..62
                for cg in range(2):
                    nc.tensor.matmul(
                        out3s[cg][:, :, 0 : IMG - 1],
                        w_tile[:, 0:IMG],
                        rhs3[:, gss[cg], 0 : IMG - 1],
                        start=False,
                        stop=False,
                        tile_position=(0, cg * IMG),
                        skip_group_check=True,
                    )
                # tap1 at j=1 accumulates
                for cg in range(2):
                    nc.tensor.matmul(
                        out3s[cg][:, :, 1:2],
                        w_tile[0:IMG, 192:256],
                        rhs3[0:IMG, gss[cg], 0:1],
                        start=False,
                        stop=False,
                        tile_position=(0, cg * IMG),
                        skip_group_check=True,
                    )
                # pair(0,1): accumulates into j=2..63
                for cg in range(2):
                    nc.tensor.matmul(
                        out3s[cg][:, :, 2:IMG],
                        w_tile[:, IMG:128],
                        rhs3[:, gss[cg], 0 : IMG - 2],
                        start=False,
                        stop=True,
                        tile_position=(0, cg * IMG),
                        skip_group_check=True,
                    )

                out_tile = opool.tile([128, 512], IN_DT)
                if b % 2 == 0:
                    nc.vector.tensor_copy(out_tile[:], ps[:])
                    nc.sync.dma_start(o_d[b], out_tile[:])
                else:
                    nc.scalar.copy(out_tile[:], ps[:])
                    nc.scalar.dma_start(o_d[b], out_tile[:])
    nc.compile()
    return nc


def _host_pack(x: np.ndarray) -> np.ndarray:
    """FULL x (8192,64,64) f32 -> [N_CORES, N_BATCH, 64, 1024] bf16.

    Partition dim = h; free dim = (g: 16 images, w: 64), tightly packed."""
    v = x.reshape(N_CORES, N_BATCH, GROUP, IMG, IMG).transpose(0, 1, 3, 2, 4)
    return np.ascontiguousarray(v.astype(NP_IN)).reshape(
        N_CORES, N_BATCH, IMG, TILE_W
    )


def _host_unpack(tiles: np.ndarray) -> np.ndarray:
    """[N_CORES, N_BATCH, 128, 512] bf16 -> (8192, 64, 64) f32.

    Partition dim = (cg, i); free dim = (g: 8, j); img = b*16 + cg*8 + g."""
    v = tiles.reshape(N_CORES, N_BATCH, 2, IMG, 8, IMG)
    v = v.transpose(0, 1, 2, 4, 3, 5)  # [core, b, cg, g, i, j]
    return v.reshape(N_IMAGES, IMG, IMG).astype(np.float32)


def kernel(x: np.ndarray, kernel: np.ndarray, _trace: bool = False) -> np.ndarray:
    global LAST_RESULTS
    x = np.ascontiguousarray(np.asarray(x, dtype=np.float32))
    n, c, h, w = x.shape
    assert (n, c, h, w) == (16, 512, 64, 64), x.shape

    shards = _host_pack(x.reshape(N_IMAGES, IMG, IMG))
    wts = _build_weights(kernel)
    in_maps = [{"x": shards[i], "wts": wts} for i in range(N_CORES)]

    nc = _bass_module()
    results = run_bass_kernel_spmd(
        nc, in_maps, core_ids=list(range(N_CORES)), trace=_trace
    )
    LAST_RESULTS = results

    tiles = np.stack([r["out"] for r in results.results])
    out = _host_unpack(tiles)
    return np.ascontiguousarray(out.reshape(n, c, h, w)).astype(np.float32)


# revision 19
# speedup vs baseline: 1.0793x; 1.0793x over previous
"""Trainium2 Bass kernel for nn_Blur (upfirdn2d 4x4 blur, pad=(2,1)).

Formulation: out[i,j] = sum_{p,q} Kf[p,q] * x[i+p-2, j+q-2]   (Kf = flip(kernel2d))

For each W-tap q (4 taps), the H-convolution is a banded 64x64 matrix
Aq[i,h] = Kf[h-i+2, q].  Tolerance is 2e-2, so x streams as a single bf16
(the {1,3,9}/64 blur weights have <=4 mantissa bits: every bf16 product is
exact in fp32; end-to-end error ~5e-3) -- half the HBM traffic of an
fp32-faithful hi/lo split.

W-taps are fused in PAIRS into the K=128 contraction: x rows live in
partitions 0-63 (16 images x 64 cols, tight 128B stride -- the PE rhs
fetcher needs the power-of-two group stride to stream at 1 col/cycle);
one SBUF->SBUF queue DMA writes the same rows shifted left one column
into partitions 64-127.  Then
  pair(2,3): lhsT=[A2^T;A3^T], rhs c=0..62, out j=c   (tap3 reads dup)
  pair(0,1): lhsT=[A0^T;A1^T], rhs c=0..61, out j=c+2 (tap1 reads dup)
plus two N=1/image boundary matmuls (tap2@j=63, tap1@j=1, K=64 on the x
rows).  PSUM per-element has_written semantics: the FIRST matmul per
column group carries the only start=True (clearing the has-written state
across its partition range); every later matmul uses start=False, which
stores where clear and accumulates where set.  Two column groups run
concurrently on disjoint PE columns (tile_position (0,0)/(0,64)).
Tensor cost: ~1020 streamed cols/batch vs 2016 for a 4-tap scheme.

Input DMAs are issued 3 batches ahead on the sync queue so out-DMA
copy-waits never starve input issue (a stall >1.5us demotes the PE
clock-boost (HAM) to 1.2 GHz, which never recovers at <3.4us of
sustained activity).  The fp32 PSUM result is copied to SBUF as bf16
(alternating vector/scalar), DMA'd back, and cast to f32 on the host.
HBM per core: 8.4 MB in + 8.4 MB out = the ~47us roofline.

Sharding: the 16*512 = 8192 independent (n,c) images are split into 8
contiguous slabs of 1024 images, one per NeuronCore (data-parallel).
"""

import ml_dtypes
import numpy as np

import concourse.bacc as bacc
import concourse.bass as bass
import concourse.mybir as mybir
import concourse.tile as tile
from concourse.bass_utils import run_bass_kernel_spmd

N_CORES = 8
IMG = 64                      # H = W
N_IMAGES = 16 * 512           # 8192
PER_CORE = N_IMAGES // N_CORES  # 1024
GROUP = 16                    # images per batch
N_BATCH = PER_CORE // GROUP   # 64
TILE_W = GROUP * IMG          # 1024 free cols, 128B image stride
DT = mybir.dt.float32
IN_DT = mybir.dt.bfloat16
NP_IN = ml_dtypes.bfloat16

LAST_RESULTS = None  # BassKernelResults of the most recent run (for test.py)


def _build_weights(kernel2d: np.ndarray) -> np.ndarray:
    """[128, 256] bf16: cols 0:64=[A2^T;A3^T], 64:128=[A0^T;A1^T],
    128:192=[A2^T;0] (tap2 boundary), 192:256=[A1^T;0] (tap1 boundary)."""
    kf = np.flip(np.asarray(kernel2d, dtype=np.float64), (0, 1))
    a = np.zeros((4, IMG, IMG), dtype=np.float64)
    for q in range(4):
        for i in range(IMG):
            for p in range(4):
                h = i + p - 2
                if 0 <= h < IMG:
                    a[q, i, h] = kf[p, q]
    wts = np.zeros((128, 256), dtype=NP_IN)
    wts[:IMG, 0:IMG] = a[2].T.astype(NP_IN)
    wts[IMG:, 0:IMG] = a[3].T.astype(NP_IN)
    wts[:IMG, IMG:128] = a[0].T.astype(NP_IN)
    wts[IMG:, IMG:128] = a[1].T.astype(NP_IN)
    wts[:IMG, 128:192] = a[2].T.astype(NP_IN)
    wts[:IMG, 192:256] = a[1].T.astype(NP_IN)
    return wts


def _bass_module() -> bass.Bass:
    nc = bacc.Bacc(
        "TRN2",
        target_bir_lowering=False,
        debug=False,
        num_devices=N_CORES,
    )
    x_d = nc.dram_tensor("x", [N_BATCH, IMG, TILE_W], IN_DT, kind="ExternalInput")
    w_d = nc.dram_tensor("wts", [128, 256], IN_DT, kind="ExternalInput")
    o_d = nc.dram_tensor("out", [N_BATCH, 128, 512], IN_DT, kind="ExternalOutput")

    with tile.TileContext(nc) as tc:
        with (
            tc.tile_pool(name="const", bufs=1) as cpool,
            tc.tile_pool(name="inp", bufs=14) as ipool,
            tc.tile_pool(name="outp", bufs=8) as opool,
            tc.tile_pool(name="psum", bufs=8, space="PSUM") as ppool,
        ):
            w_tile = cpool.tile([128, 256], IN_DT)
            nc.sync.dma_start(w_tile[:], w_d[:])

            # HAM warmup: the PE clock-gate holds 1.2 GHz until ~3.4us of
            # sustained matmul activity.  Burn that window on dummy matmuls
            # (zeroed operands, result never read) that overlap the first
            # input DMA, so the real matmuls start at full clock.
            dummy = cpool.tile([128, 512], IN_DT, tag="warm_sbuf")
            nc.gpsimd.memset(dummy[:], 0.0)
            warm_ps = ppool.tile([128, 512], DT, tag="ps")
            for _ in range(8):
                nc.tensor.matmul(
                    warm_ps[:], dummy[:, 0:128], dummy[:], start=True, stop=True
                )

            def issue_in(b):
                t = ipool.tile([128, TILE_W], IN_DT)
                nc.sync.dma_start(t[0:IMG, :], x_d[b])
                in_tiles[b] = t

            def issue_dup(b):
                # shifted dup: partitions 64-127 = x rows one col left.
                # Per-image col 63 holds next-image garbage; no matmul
                # window ever reads it.  Issued IN_AHEAD batches after the
                # input DMA so its wait on in_b is already satisfied and
                # never blocks the input stream on the sync queue.
                t = in_tiles[b]
                nc.sync.dma_start(
                    t[IMG:128, 0 : TILE_W - 1], t[0:IMG, 1:TILE_W]
                )

            IN_AHEAD = 12
            in_tiles = {}
            for b in range(IN_AHEAD):
                issue_in(b)
            issue_dup(0)
            issue_dup(1)

            for b in range(N_BATCH):
                if b + IN_AHEAD < N_BATCH:
                    issue_in(b + IN_AHEAD)
                if b + 2 < N_BATCH:
                    issue_dup(b + 2)
                in_tile = in_tiles.pop(b)
                rhs3 = in_tile[:].rearrange("p (g w) -> p g w", w=IMG)

                ps = ppool.tile([128, 512], DT)
                out3s = [
                    ps[cg * IMG : (cg + 1) * IMG, :].rearrange(
                        "p (g w) -> p g w", w=IMG
                    )
                    for cg in range(2)
                ]
                gss = [slice(cg * 8, (cg + 1) * 8) for cg in range(2)]
                # tap2 at j=63 goes FIRST: its start=True initializes the
                # per-element has_written state for the whole column group.
                for cg in range(2):
                    nc.tensor.matmul(
                        out3s[cg][:, :, IMG - 1 : IMG],
                        w_tile[0:IMG, 128:192],
                        rhs3[0:IMG, gss[cg], IMG - 1 : IMG],
                        start=True,
                        stop=False,
                        tile_position=(0, cg * IMG),
                        skip_group_check=True,
                    )
                # pair(2,3): stores into cleared elements j=0..62
                for cg in range(2):
                    nc.tensor.matmul(
                        out3s[cg][:, :, 0 : IMG - 1],
                        w_tile[:, 0:IMG],
                        rhs3[:, gss[cg], 0 : IMG - 1],
                        start=False,
                        stop=False,
                        tile_position=(0, cg * IMG),
                        skip_group_check=True,
                    )
                # tap1 at j=1 accumulates
                for cg in range(2):
                    nc.tensor.matmul(
                        out3s[cg][:, :, 1:2],
                        w_tile[0:IMG, 192:256],
                        rhs3[0:IMG, gss[cg], 0:1],
                        start=False,
                        stop=False,
                        tile_position=(0, cg * IMG),
                        skip_group_check=True,
                    )
                # pair(0,1): accumulates into j=2..63
                for cg in range(2):
                    nc.tensor.matmul(
                        out3s[cg][:, :, 2:IMG],
                        w_tile[:, IMG:128],
                        rhs3[:, gss[cg], 0 : IMG - 2],
                        start=False,
                        stop=True,
                        tile_position=(0, cg * IMG),
                        skip_group_check=True,
                    )

                out_tile = opool.tile([128, 512], IN_DT)
                if b % 2 == 0:
                    nc.vector.tensor_copy(out_tile[:], ps[:])
                    nc.sync.dma_start(o_d[b], out_tile[:])
                else:
                    nc.scalar.copy(out_tile[:], ps[:])
                    nc.scalar.dma_start(o_d[b], out_tile[:])
    nc.compile()
    return nc


def _host_pack(x: np.ndarray) -> np.ndarray:
    """FULL x (8192,64,64) f32 -> [N_CORES, N_BATCH, 64, 1024] bf16.

    Partition dim = h; free dim = (g: 16 images, w: 64), tightly packed."""
    v = x.reshape(N_CORES, N_BATCH, GROUP, IMG, IMG).transpose(0, 1, 3, 2, 4)
    return np.ascontiguousarray(v.astype(NP_IN)).reshape(
        N_CORES, N_BATCH, IMG, TILE_W
    )


def _host_unpack(tiles: np.ndarray) -> np.ndarray:
    """[N_CORES, N_BATCH, 128, 512] bf16 -> (8192, 64, 64) f32.

    Partition dim = (cg, i); free dim = (g: 8, j); img = b*16 + cg*8 + g."""
    v = tiles.reshape(N_CORES, N_BATCH, 2, IMG, 8, IMG)
    v = v.transpose(0, 1, 2, 4, 3, 5)  # [core, b, cg, g, i, j]
    return v.reshape(N_IMAGES, IMG, IMG).astype(np.float32)


def kernel(x: np.ndarray, kernel: np.ndarray, _trace: bool = False) -> np.ndarray:
    global LAST_RESULTS
    x = np.ascontiguousarray(np.asarray(x, dtype=np.float32))
    n, c, h, w = x.shape
    assert (n, c, h, w) == (16, 512, 64, 64), x.shape

    shards = _host_pack(x.reshape(N_IMAGES, IMG, IMG))
    wts = _build_weights(kernel)
    in_maps = [{"x": shards[i], "wts": wts} for i in range(N_CORES)]

    nc = _bass_module()
    results = run_bass_kernel_spmd(
        nc, in_maps, core_ids=list(range(N_CORES)), trace=_trace
    )
    LAST_RESULTS = results

    tiles = np.stack([r["out"] for r in results.results])
    out = _host_unpack(tiles)
    return np.ascontiguousarray(out.reshape(n, c, h, w)).astype(np.float32)


# revision 20
# speedup vs baseline: 1.2508x; 1.1589x over previous
"""Trainium2 Bass kernel for nn_Blur (upfirdn2d 4x4 blur, pad=(2,1)).

Formulation: out[i,j] = sum_{p,q} Kf[p,q] * x[i+p-2, j+q-2]   (Kf = flip(kernel2d))

For each W-tap q (4 taps), the H-convolution is a banded 64x64 matrix
Aq[i,h] = Kf[h-i+2, q].  Tolerance is 2e-2, so x streams as a single bf16
(the {1,3,9}/64 blur weights have <=4 mantissa bits: every bf16 product is
exact in fp32; end-to-end error ~5e-3) -- half the HBM traffic of an
fp32-faithful hi/lo split.

W-taps are fused in PAIRS into the K=128 contraction: x rows live in
partitions 0-63 (16 images x 64 cols, tight 128B stride -- the PE rhs
fetcher needs the power-of-two group stride to stream at 1 col/cycle);
one SBUF->SBUF queue DMA writes the same rows shifted left one column
into partitions 64-127.  Then
  pair(2,3): lhsT=[A2^T;A3^T], rhs c=0..62, out j=c   (tap3 reads dup)
  pair(0,1): lhsT=[A0^T;A1^T], rhs c=0..61, out j=c+2 (tap1 reads dup)
plus two N=1/image boundary matmuls (tap2@j=63, tap1@j=1, K=64 on the x
rows).  PSUM per-element has_written semantics: the FIRST matmul per
column group carries the only start=True (clearing the has-written state
across its partition range); every later matmul uses start=False, which
stores where clear and accumulates where set.  Two column groups run
concurrently on disjoint PE columns (tile_position (0,0)/(0,64)).
Tensor cost: ~1020 streamed cols/batch vs 2016 for a 4-tap scheme.

Input DMAs are issued 3 batches ahead on the sync queue so out-DMA
copy-waits never starve input issue (a stall >1.5us demotes the PE
clock-boost (HAM) to 1.2 GHz, which never recovers at <3.4us of
sustained activity).  The fp32 PSUM result is copied to SBUF as bf16
(alternating vector/scalar), DMA'd back, and cast to f32 on the host.
HBM per core: 8.4 MB in + 8.4 MB out = the ~47us roofline.

Sharding: the 16*512 = 8192 independent (n,c) images are split into 8
contiguous slabs of 1024 images, one per NeuronCore (data-parallel).
"""

import ml_dtypes
import numpy as np

import concourse.bacc as bacc
import concourse.bass as bass
import concourse.mybir as mybir
import concourse.tile as tile
from concourse.bass_utils import run_bass_kernel_spmd

N_CORES = 8
IMG = 64                      # H = W
N_IMAGES = 16 * 512           # 8192
PER_CORE = N_IMAGES // N_CORES  # 1024
GROUP = 16                    # images per batch
N_BATCH = PER_CORE // GROUP   # 64
TILE_W = GROUP * IMG          # 1024 free cols, 128B image stride
DT = mybir.dt.float32
IN_DT = mybir.dt.bfloat16
NP_IN = ml_dtypes.bfloat16

LAST_RESULTS = None  # BassKernelResults of the most recent run (for test.py)


def _build_weights(kernel2d: np.ndarray) -> np.ndarray:
    """[128, 256] bf16: cols 0:64=[A2^T;A3^T], 64:128=[A0^T;A1^T],
    128:192=[A2^T;0] (tap2 boundary), 192:256=[A1^T;0] (tap1 boundary)."""
    kf = np.flip(np.asarray(kernel2d, dtype=np.float64), (0, 1))
    a = np.zeros((4, IMG, IMG), dtype=np.float64)
    for q in range(4):
        for i in range(IMG):
            for p in range(4):
                h = i + p - 2
                if 0 <= h < IMG:
                    a[q, i, h] = kf[p, q]
    wts = np.zeros((128, 256), dtype=NP_IN)
    wts[:IMG, 0:IMG] = a[2].T.astype(NP_IN)
    wts[IMG:, 0:IMG] = a[3].T.astype(NP_IN)
    wts[:IMG, IMG:128] = a[0].T.astype(NP_IN)
    wts[IMG:, IMG:128] = a[1].T.astype(NP_IN)
    wts[:IMG, 128:192] = a[2].T.astype(NP_IN)
    wts[:IMG, 192:256] = a[1].T.astype(NP_IN)
    return wts


def _bass_module() -> bass.Bass:
    nc = bacc.Bacc(
        "TRN2",
        target_bir_lowering=False,
        debug=False,
        num_devices=N_CORES,
    )
    x_d = nc.dram_tensor("x", [N_BATCH, IMG, TILE_W], IN_DT, kind="ExternalInput")
    w_d = nc.dram_tensor("wts", [128, 256], IN_DT, kind="ExternalInput")
    o_d = nc.dram_tensor("out", [N_BATCH, 128, 512], IN_DT, kind="ExternalOutput")

    with tile.TileContext(nc) as tc:
        with (
            tc.tile_pool(name="const", bufs=1) as cpool,
            tc.tile_pool(name="inp", bufs=14) as ipool,
            tc.tile_pool(name="outp", bufs=8) as opool,
            tc.tile_pool(name="psum", bufs=8, space="PSUM") as ppool,
        ):
            w_tile = cpool.tile([128, 256], IN_DT)
            nc.sync.dma_start(w_tile[:], w_d[:])

            # HAM warmup: the PE clock-gate holds 1.2 GHz until ~3.4us of
            # sustained matmul activity.  Burn that window on dummy matmuls
            # (zeroed operands, result never read) that overlap the first
            # input DMA, so the real matmuls start at full clock.
            dummy = cpool.tile([128, 512], IN_DT, tag="warm_sbuf")
            nc.gpsimd.memset(dummy[:], 0.0)
            warm_ps = ppool.tile([128, 512], DT, tag="ps")
            for _ in range(8):
                nc.tensor.matmul(
                    warm_ps[:], dummy[:, 0:128], dummy[:], start=True, stop=True
                )

            def issue_in(b):
                t = ipool.tile([128, TILE_W], IN_DT)
                nc.sync.dma_start(t[0:IMG, :], x_d[b])
                in_tiles[b] = t

            def issue_dup(b):
                # shifted dup: partitions 64-127 = x rows one col left.
                # Per-image col 63 holds next-image garbage; no matmul
                # window ever reads it.  Issued IN_AHEAD batches after the
                # input DMA so its wait on in_b is already satisfied and
                # never blocks the input stream on the sync queue.
                t = in_tiles[b]
                nc.sync.dma_start(
                    t[IMG:128, 0 : TILE_W - 1], t[0:IMG, 1:TILE_W]
                )

            IN_AHEAD = 12
            DUP_AHEAD = 6
            in_tiles = {}
            for b in range(IN_AHEAD):
                issue_in(b)
            for b in range(DUP_AHEAD):
                issue_dup(b)

            for b in range(N_BATCH):
                if b + IN_AHEAD < N_BATCH:
                    issue_in(b + IN_AHEAD)
                if b + DUP_AHEAD < N_BATCH:
                    issue_dup(b + DUP_AHEAD)
                in_tile = in_tiles.pop(b)
                rhs3 = in_tile[:].rearrange("p (g w) -> p g w", w=IMG)

                ps = ppool.tile([128, 512], DT)
                out3s = [
                    ps[cg * IMG : (cg + 1) * IMG, :].rearrange(
                        "p (g w) -> p g w", w=IMG
                    )
                    for cg in range(2)
                ]
                gss = [slice(cg * 8, (cg + 1) * 8) for cg in range(2)]
                # tap2 at j=63 goes FIRST: its start=True initializes the
                # per-element has_written state for the whole column group.
                for cg in range(2):
                    nc.tensor.matmul(
                        out3s[cg][:, :, IMG - 1 : IMG],
                        w_tile[0:IMG, 128:192],
                        rhs3[0:IMG, gss[cg], IMG - 1 : IMG],
                        start=True,
                        stop=False,
                        tile_position=(0, cg * IMG),
                        skip_group_check=True,
                    )
                # pair(2,3): stores into cleared elements j=0..62
                for cg in range(2):
                    nc.tensor.matmul(
                        out3s[cg][:, :, 0 : IMG - 1],
                        w_tile[:, 0:IMG],
                        rhs3[:, gss[cg], 0 : IMG - 1],
                        start=False,
                        stop=False,
                        tile_position=(0, cg * IMG),
                        skip_group_check=True,
                    )
                # tap1 at j=1 accumulates
                for cg in range(2):
                    nc.tensor.matmul(
                        out3s[cg][:, :, 1:2],
                        w_tile[0:IMG, 192:256],
                        rhs3[0:IMG, gss[cg], 0:1],
                        start=False,
                        stop=False,
                        tile_position=(0, cg * IMG),
                        skip_group_check=True,
                    )
                # pair(0,1): accumulates into j=2..63
                for cg in range(2):
                    nc.tensor.matmul(
                        out3s[cg][:, :, 2:IMG],
                        w_tile[:, IMG:128],
                        rhs3[:, gss[cg], 0 : IMG - 2],
                        start=False,
                        stop=True,
                        tile_position=(0, cg * IMG),
                        skip_group_check=True,
                    )

                # keep-alive dummy matmuls hold the HAM duty fraction above
                # the demote threshold: the fill phase gets a heavy bridge,
                # steady state a light one.
                n_dummy, dummy_n = (2, 512) if b < 12 else (1, 256)
                for _ in range(n_dummy):
                    nc.tensor.matmul(
                        warm_ps[:, 0:dummy_n],
                        dummy[:, 0:128],
                        dummy[:, 0:dummy_n],
                        start=True,
                        stop=True,
                    )

                out_tile = opool.tile([128, 512], IN_DT)
                if b % 2 == 0:
                    nc.vector.tensor_copy(out_tile[:], ps[:])
                    nc.sync.dma_start(o_d[b], out_tile[:])
                else:
                    nc.scalar.copy(out_tile[:], ps[:])
                    nc.scalar.dma_start(o_d[b], out_tile[:])
    nc.compile()
    return nc


def _host_pack(x: np.ndarray) -> np.ndarray:
    """FULL x (8192,64,64) f32 -> [N_CORES, N_BATCH, 64, 1024] bf16.

    Partition dim = h; free dim = (g: 16 images, w: 64), tightly packed."""
    v = x.reshape(N_CORES, N_BATCH, GROUP, IMG, IMG).transpose(0, 1, 3, 2, 4)
    return np.ascontiguousarray(v.astype(NP_IN)).reshape(
        N_CORES, N_BATCH, IMG, TILE_W
    )


def _host_unpack(tiles: np.ndarray) -> np.ndarray:
    """[N_CORES, N_BATCH, 128, 512] bf16 -> (8192, 64, 64) f32.

    Partition dim = (cg, i); free dim = (g: 8, j); img = b*16 + cg*8 + g."""
    v = tiles.reshape(N_CORES, N_BATCH, 2, IMG, 8, IMG)
    v = v.transpose(0, 1, 2, 4, 3, 5)  # [core, b, cg, g, i, j]
    return v.reshape(N_IMAGES, IMG, IMG).astype(np.float32)


def kernel(x: np.ndarray, kernel: np.ndarray, _trace: bool = False) -> np.ndarray:
    global LAST_RESULTS
    x = np.ascontiguousarray(np.asarray(x, dtype=np.float32))
    n, c, h, w = x.shape
    assert (n, c, h, w) == (16, 512, 64, 64), x.shape

    shards = _host_pack(x.reshape(N_IMAGES, IMG, IMG))
    wts = _build_weights(kernel)
    in_maps = [{"x": shards[i], "wts": wts} for i in range(N_CORES)]

    nc = _bass_module()
    results = run_bass_kernel_spmd(
        nc, in_maps, core_ids=list(range(N_CORES)), trace=_trace
    )
    LAST_RESULTS = results

    tiles = np.stack([r["out"] for r in results.results])
    out = _host_unpack(tiles)
    return np.ascontiguousarray(out.reshape(n, c, h, w)).astype(np.float32)


# revision 21
# speedup vs baseline: 1.9767x; 1.5803x over previous
"""Trainium2 Bass kernel for nn_Blur (upfirdn2d 4x4 blur, pad=(2,1)).

Formulation: out[i,j] = sum_{p,q} Kf[p,q] * x[i+p-2, j+q-2]   (Kf = flip(kernel2d))

For each W-tap q (4 taps), the H-convolution is a banded 64x64 matrix
Aq[i,h] = Kf[h-i+2, q].  Tolerance is 2e-2, so x streams as a single bf16
(the {1,3,9}/64 blur weights have <=4 mantissa bits: every bf16 product is
exact in fp32; end-to-end error ~5e-3) -- HALF the HBM traffic of an
fp32-faithful hi/lo split.

The K=128 contraction is filled by stacking TWO images per partition set:
lhsT_q = blockdiag(Aq^T, Aq^T) [128,128], rhs = [x_even; x_odd] [128, N],
so each matmul computes both images' H-conv at M=128 (full PE width, no
tile_position games).  The 4 taps accumulate into one PSUM bank with
variable-width windows: tap q=2 covers the full width first (start=True
initializes the per-element has_written state everywhere), the narrower
boundary taps then accumulate into column subsets.  This keeps the PE
~90% busy, which holds the clock-boost (HAM) state -- schemes with less
tensor work demote the PE clock to 1.2 GHz and end up slower.

The fp32 PSUM result is copied to SBUF as bf16 (alternating vector /
scalar engines), DMA'd back as [128,512] bf16 tiles, and cast to f32 on
the host.  HBM per core: 8.4 MB in + 8.4 MB out = the ~47us roofline.

Sharding: the 16*512 = 8192 independent (n,c) images are split into 8
contiguous slabs of 1024 images, one per NeuronCore (data-parallel).
"""

import ml_dtypes
import numpy as np

import concourse.bacc as bacc
import concourse.bass as bass
import concourse.mybir as mybir
import concourse.tile as tile
from concourse.bass_utils import run_bass_kernel_spmd

N_CORES = 8
IMG = 64                      # H = W
N_IMAGES = 16 * 512           # 8192
PER_CORE = N_IMAGES // N_CORES  # 1024
GROUP = 16                    # images per batch (8 pairs stacked in K)
N_BATCH = PER_CORE // GROUP   # 64
TILE_W = 8 * IMG              # 512 free cols: 8 image pairs
# per-tap W windows: tap q reads x cols [XLO[q], +LEN[q]) and writes out
# cols [JLO[q], +LEN[q]).  q=2 goes first: full width, start=True.
TAP_ORDER = (2, 0, 1, 3)
XLO = (0, 0, 0, 1)
JLO = (2, 1, 0, 0)
LEN = (62, 63, 64, 63)
DT = mybir.dt.float32
IN_DT = mybir.dt.bfloat16
NP_IN = ml_dtypes.bfloat16

LAST_RESULTS = None  # BassKernelResults of the most recent run (for test.py)


def _build_weights(kernel2d: np.ndarray) -> np.ndarray:
    """[128, 512] bf16: cols [128q, 128q+128) = blockdiag(Aq^T, Aq^T)."""
    kf = np.flip(np.asarray(kernel2d, dtype=np.float64), (0, 1))
    wts = np.zeros((128, 512), dtype=NP_IN)
    for q in range(4):
        aq = np.zeros((IMG, IMG), dtype=np.float64)
        for i in range(IMG):
            for p in range(4):
                h = i + p - 2
                if 0 <= h < IMG:
                    aq[i, h] = kf[p, q]
        aqt = aq.T.astype(NP_IN)
        wts[:IMG, 128 * q : 128 * q + IMG] = aqt
        wts[IMG:, 128 * q + IMG : 128 * q + 128] = aqt
    return wts


def _bass_module() -> bass.Bass:
    nc = bacc.Bacc(
        "TRN2",
        target_bir_lowering=False,
        debug=False,
        num_devices=N_CORES,
    )
    x_d = nc.dram_tensor("x", [N_BATCH, 128, TILE_W], IN_DT, kind="ExternalInput")
    w_d = nc.dram_tensor("wts", [128, 512], IN_DT, kind="ExternalInput")
    o_d = nc.dram_tensor("out", [N_BATCH, 128, 512], IN_DT, kind="ExternalOutput")

    with tile.TileContext(nc) as tc:
        with (
            tc.tile_pool(name="const", bufs=1) as cpool,
            tc.tile_pool(name="inp", bufs=14) as ipool,
            tc.tile_pool(name="outp", bufs=8) as opool,
            tc.tile_pool(name="psum", bufs=8, space="PSUM") as ppool,
        ):
            w_tile = cpool.tile([128, 512], IN_DT)
            nc.sync.dma_start(w_tile[:], w_d[:])

            # HAM warmup: the PE clock-gate holds 1.2 GHz until ~3.4us of
            # sustained matmul activity.  Burn that window on dummy matmuls
            # (zeroed operands, result never read) that overlap the first
            # input DMA, so the real matmuls start at full clock.
            dummy = cpool.tile([128, 512], IN_DT, tag="warm_sbuf")
            nc.gpsimd.memset(dummy[:], 0.0)
            warm_ps = ppool.tile([128, 512], DT, tag="ps")
            for _ in range(9):
                nc.tensor.matmul(
                    warm_ps[:], dummy[:, 0:128], dummy[:], start=True, stop=True
                )

            # Input DMAs issue LOOKAHEAD batches early on the sync queue, so
            # the out-DMA copy-waits (also on sync, even batches) never
            # starve input issue -- that stall opened a ~1us PE gap every
            # other batch.
            LOOKAHEAD = 12
            in_tiles = {}
            for b in range(LOOKAHEAD):
                t = ipool.tile([128, TILE_W], IN_DT)
                nc.sync.dma_start(t[:], x_d[b])
                in_tiles[b] = t

            for b in range(N_BATCH):
                if b + LOOKAHEAD < N_BATCH:
                    t = ipool.tile([128, TILE_W], IN_DT)
                    nc.sync.dma_start(t[:], x_d[b + LOOKAHEAD])
                    in_tiles[b + LOOKAHEAD] = t
                in_tile = in_tiles.pop(b)
                rhs3 = in_tile[:].rearrange("p (g w) -> p g w", w=IMG)

                ps = ppool.tile([128, 512], DT)
                out3 = ps[:].rearrange("p (g w) -> p g w", w=IMG)
                for qi, q in enumerate(TAP_ORDER):
                    nc.tensor.matmul(
                        out3[:, :, JLO[q] : JLO[q] + LEN[q]],
                        w_tile[:, 128 * q : 128 * q + 128],
                        rhs3[:, :, XLO[q] : XLO[q] + LEN[q]],
                        start=(qi == 0),
                        stop=(qi == 3),
                    )

                out_tile = opool.tile([128, 512], IN_DT)
                if b % 2 == 0:
                    nc.vector.tensor_copy(out_tile[:], ps[:])
                    nc.sync.dma_start(o_d[b], out_tile[:])
                else:
                    nc.scalar.copy(out_tile[:], ps[:])
                    nc.scalar.dma_start(o_d[b], out_tile[:])
    nc.compile()
    return nc


def _host_pack(x: np.ndarray) -> np.ndarray:
    """FULL x (8192,64,64) f32 -> [N_CORES, N_BATCH, 128, 512] bf16.

    Partition dim = (a, h), a = image parity in pair; free dim = (g: 8
    pairs, w).  img = core*1024 + b*16 + g*2 + a."""
    v = x.reshape(N_CORES, N_BATCH, 8, 2, IMG, IMG).transpose(0, 1, 3, 4, 2, 5)
    return np.ascontiguousarray(v.astype(NP_IN)).reshape(
        N_CORES, N_BATCH, 128, TILE_W
    )


def _host_unpack(tiles: np.ndarray) -> np.ndarray:
    """[N_CORES, N_BATCH, 128, 512] bf16 -> (8192, 64, 64) f32."""
    v = tiles.reshape(N_CORES, N_BATCH, 2, IMG, 8, IMG)
    v = v.transpose(0, 1, 4, 2, 3, 5)  # [core, b, g, a, i, j]
    return v.reshape(N_IMAGES, IMG, IMG).astype(np.float32)


def kernel(x: np.ndarray, kernel: np.ndarray, _trace: bool = False) -> np.ndarray:
    global LAST_RESULTS
    x = np.ascontiguousarray(np.asarray(x, dtype=np.float32))
    n, c, h, w = x.shape
    assert (n, c, h, w) == (16, 512, 64, 64), x.shape

    shards = _host_pack(x.reshape(N_IMAGES, IMG, IMG))
    wts = _build_weights(kernel)
    in_maps = [{"x": shards[i], "wts": wts} for i in range(N_CORES)]

    nc = _bass_module()
    results = run_bass_kernel_spmd(
        nc, in_maps, core_ids=list(range(N_CORES)), trace=_trace
    )
    LAST_RESULTS = results

    tiles = np.stack([r["out"] for r in results.results])
    out = _host_unpack(tiles)
    return np.ascontiguousarray(out.reshape(n, c, h, w)).astype(np.float32)
